# revision 1
# baseline (speedup 1.0000x reference)
"""BiLSTM Trainium2 kernel.

Sharding: 8 cores = 4 batch quarters x 2 directions.
  core p: direction d = p // 4 (0=fwd, 1=bwd), batch quarter q = p % 4
  (the backward direction is the forward LSTM run on a time-reversed
  sequence; the final reduction is a max over time, which is order-invariant,
  so all 8 cores run the identical program on different data.)

Per core: 3 stacked LSTM layers over T steps, batch 32, H=256, run as a
lag-1 wavefront (layer l processes step t = tick - l), fully SBUF-resident:
  - token embeddings gathered from HBM via indirect DMA, PE-transposed into a
    feature-major X^T buffer (bf16)
  - per tick: matmuls (weights streaming, batch-on-partition, fp32 PSUM accum)
    -> fused sigmoid/tanh on ScalarE across all active layers
    -> DVE cell-state update -> tanh(c) -> h -> PE transpose of h into
    feature-major h^T (the lhsT of the next tick's matmuls)
  - running max over t of layer-2 h^T
Final dense layers run on every core after an AllGather of the per-core maxes;
the host takes core 0's output.

Gate columns are permuted on host from TF order [i,j,f,o] to [f,i,o,j] so a
single ScalarE sigmoid covers all three sigmoid gates; when the layer-1/2
biases are all zero (the usual case) the +1.0 forget bias is applied for free
via the ScalarE activation-bias field and no per-step bias matmuls are
emitted; otherwise biases ride in an extra weight row against a ones-vector.
cap_table is folded into the layer-0 weights (one-hot @ (cap_table @ W_cap)).
"""

import sys

import numpy as np

sys.path.insert(0, "/opt/trn_rl_repo")

from contextlib import ExitStack

import concourse.bacc as bacc
import concourse.bass as bass
import concourse.mybir as mybir
import concourse.tile as tile
from concourse.bass import IndirectOffsetOnAxis
from concourse.bass_utils import run_bass_kernel_spmd
from concourse.masks import make_identity

FP32 = mybir.dt.float32
BF16 = mybir.dt.bfloat16
INT32 = mybir.dt.int32

VOCAB, EMB, T_FULL, B_FULL, H, NC_OUT = 50000, 200, 500, 128, 256, 6
BQ = 32          # batch per core
G4 = 4 * H       # 1024 gate width
HALF = 512       # matmul N per PSUM bank

# gate slices after host permutation [f, i, o, j]
SL_F = slice(0, 256)
SL_I = slice(256, 512)
SL_O = slice(512, 768)
SL_J = slice(768, 1024)


def _build_program(T, with_tail=True, has_bias=True):
    """Build the single SPMD Bass program (same for every core)."""
    TOK = BQ * T                      # tokens per core
    NTILE = TOK // 128                # 128-token gather tiles
    assert TOK % 128 == 0

    nc = bacc.Bacc(None, target_bir_lowering=False, debug=False)

    # ---- external inputs (per-core data) ----
    widx = nc.dram_tensor("widx", [128, NTILE], INT32, kind="ExternalInput")
    caph = nc.dram_tensor("caph", [5, TOK], BF16, kind="ExternalInput")
    emb = nc.dram_tensor("emb", [VOCAB, EMB], FP32, kind="ExternalInput")
    w0 = nc.dram_tensor("w0", [461, G4], BF16, kind="ExternalInput")
    wrows = 513 if has_bias else 512
    w1 = nc.dram_tensor("w1", [wrows, G4], BF16, kind="ExternalInput")
    w2 = nc.dram_tensor("w2", [wrows, G4], BF16, kind="ExternalInput")
    d1w = nc.dram_tensor("d1w", [512, 64], BF16, kind="ExternalInput")
    d1b = nc.dram_tensor("d1b", [1, 64], BF16, kind="ExternalInput")
    d2w = nc.dram_tensor("d2w", [64, NC_OUT], FP32, kind="ExternalInput")
    d2b = nc.dram_tensor("d2b", [1, NC_OUT], FP32, kind="ExternalInput")
    out = nc.dram_tensor("out", [NC_OUT, B_FULL], FP32, kind="ExternalOutput")

    with tile.TileContext(nc) as tc, ExitStack() as ctx:
        const = ctx.enter_context(tc.tile_pool(name="const", bufs=1))
        wpool = ctx.enter_context(tc.tile_pool(name="wpool", bufs=1))
        xtp = ctx.enter_context(tc.tile_pool(name="xtp", bufs=1))
        state = ctx.enter_context(tc.tile_pool(name="state", bufs=1))
        gpool = ctx.enter_context(tc.tile_pool(name="gpool", bufs=3))
        zg = ctx.enter_context(tc.tile_pool(name="zg", bufs=3))
        hpool = ctx.enter_context(tc.tile_pool(name="hpool", bufs=2))
        htp = ctx.enter_context(tc.tile_pool(name="htp", bufs=2))
        dram = ctx.enter_context(tc.tile_pool(name="dram", bufs=1, space="DRAM"))

        # ---- constants ----
        id_f32 = const.tile([128, 128], FP32)
        make_identity(nc, id_f32[:])
        id_bf = const.tile([128, 128], BF16)
        nc.vector.tensor_copy(id_bf[:], id_f32[:])
        ones_bf = const.tile([1, 128], BF16)
        nc.gpsimd.memset(ones_bf[:], 1.0)
        ones_f32 = const.tile([1, 128], FP32)
        nc.gpsimd.memset(ones_f32[:], 1.0)

        # ---- load weights into SBUF ----
        def load_w(dw, rows_chunks):
            tiles = []
            r0 = 0
            for i, rs in enumerate(rows_chunks):
                t = wpool.tile([rs, G4], BF16, name=f"wt_{dw.name}_{i}")
                nc.sync.dma_start(t[:], dw[r0:r0 + rs, :])
                tiles.append(t)
                r0 += rs
            return tiles

        w0a, w0b, w0c, w0d = load_w(w0, [128, 77, 128, 128])
        if has_bias:
            w1a, w1b, w1bias, w1c, w1d = load_w(w1, [128, 128, 1, 128, 128])
            w2a, w2b, w2bias, w2c, w2d = load_w(w2, [128, 128, 1, 128, 128])
        else:
            w1a, w1b, w1c, w1d = load_w(w1, [128, 128, 128, 128])
            w2a, w2b, w2c, w2d = load_w(w2, [128, 128, 128, 128])
            w1bias = w2bias = None

        d1w_sb = []
        for c in range(4):
            t = wpool.tile([128, 64], BF16, name=f"d1w_{c}")
            nc.sync.dma_start(t[:], d1w[128 * c:128 * (c + 1), :])
            d1w_sb.append(t)
        d1b_sb = wpool.tile([1, 64], BF16)
        nc.sync.dma_start(d1b_sb[:], d1b[:, :])
        d2w_sb = wpool.tile([64, NC_OUT], FP32)
        nc.sync.dma_start(d2w_sb[:], d2w[:, :])
        d2b_sb = wpool.tile([1, NC_OUT], FP32)
        nc.sync.dma_start(d2b_sb[:], d2b[:, :])

        # ---- recurrent state ----
        c_all = state.tile([96, H], FP32)       # cell state, 3 layers x 32 batch
        nc.gpsimd.memset(c_all[:], 0.0)
        maxht = state.tile([128, 2, BQ], BF16)  # running max of layer-2 h^T
        nc.gpsimd.memset(maxht[:], -10.0)
        ht_init = state.tile([128, 2, 96], BF16)
        nc.gpsimd.memset(ht_init[:], 0.0)

        # X^T: xt_a rows = emb features 0:128
        #      xt_b rows = emb features 128:200 (72) | cap one-hot (4) | ones (1)
        xt_a = xtp.tile([128, TOK], BF16)
        xt_b = xtp.tile([77, TOK], BF16)
        nc.sync.dma_start(xt_b[72:77, :], caph[:, :])

        widx_sb = const.tile([128, NTILE], INT32)
        nc.sync.dma_start(widx_sb[:], widx[:, :])

        with tc.tile_pool(name="pprep", bufs=2, space="PSUM") as pprep, \
             tc.tile_pool(name="pz", bufs=2, space="PSUM") as pz, \
             tc.tile_pool(name="pht", bufs=2, space="PSUM") as pht:

            # ---- embedding gather + transpose into X^T ----
            for j in range(NTILE):
                g = gpool.tile([128, EMB], FP32, name="gemb", tag="gemb")
                nc.gpsimd.indirect_dma_start(
                    out=g[:],
                    out_offset=None,
                    in_=emb[:, :],
                    in_offset=IndirectOffsetOnAxis(ap=widx_sb[:, j:j + 1], axis=0),
                )
                g2 = gpool.tile([128, EMB], BF16, name="gemb2", tag="gemb2")
                nc.gpsimd.tensor_copy(g2[:], g[:])
                tp1 = pprep.tile([128, 128], BF16, name="tp1", tag="tp")
                nc.tensor.transpose(tp1[:], g2[:, 0:128], id_bf[:])
                nc.vector.tensor_copy(xt_a[:, 128 * j:128 * (j + 1)], tp1[:])
                tp2 = pprep.tile([72, 128], BF16, name="tp2", tag="tp")
                nc.tensor.transpose(tp2[:], g2[:, 128:200], id_bf[:])
                nc.vector.tensor_copy(xt_b[0:72, 128 * j:128 * (j + 1)], tp2[:])

            ht_prev = ht_init

            # per-layer lhsT chunk lists for step t of layer l
            def layer_chunks(l, t, ht):
                if l == 0:
                    return [
                        (xt_a[:, BQ * t:BQ * (t + 1)], w0a),
                        (xt_b[:, BQ * t:BQ * (t + 1)], w0b),
                        (ht[:, 0, 0:32], w0c),
                        (ht[:, 1, 0:32], w0d),
                    ]
                wa, wb, wbias, wc, wd = (
                    (w1a, w1b, w1bias, w1c, w1d) if l == 1 else
                    (w2a, w2b, w2bias, w2c, w2d))
                xs = slice(32 * (l - 1), 32 * l)
                hs = slice(32 * l, 32 * (l + 1))
                chunks = [
                    (ht[:, 0, xs], wa),
                    (ht[:, 1, xs], wb),
                    (ht[:, 0, hs], wc),
                    (ht[:, 1, hs], wd),
                ]
                if has_bias:
                    chunks.insert(2, (ones_bf[0:1, 0:32], wbias))
                return chunks

            # L0's x-part matmuls depend only on X^T; emit tick tau+1's
            # before tick tau's transposes so the in-order PE fills its
            # stall window while the ACT/DVE tail of tick tau runs
            z_tiles = {}

            def alloc_z(tau):
                zt = pz.tile([96, G4], FP32, name="z", tag="z")
                z_tiles[tau] = zt
                if tau <= T - 1:
                    for half in range(2):
                        ns = slice(HALF * half, HALF * (half + 1))
                        for k, lhsT in enumerate(
                                (xt_a[:, BQ * tau:BQ * (tau + 1)],
                                 xt_b[:, BQ * tau:BQ * (tau + 1)])):
                            rhs = (w0a, w0b)[k]
                            nc.tensor.matmul(
                                zt[0:32, ns], lhsT, rhs[:, ns],
                                start=(k == 0), stop=False,
                                skip_group_check=True)
                return zt

            alloc_z(0)

            # ---- wavefront over ticks ----
            for tau in range(T + 2):
                lo = max(0, tau - (T - 1))
                hi = min(2, tau)
                # HW: a partition range with non-zero base spans <= 32
                if lo == 0:
                    rlist = [slice(0, 32 * (hi + 1))]
                else:
                    rlist = [slice(32 * l, 32 * (l + 1))
                             for l in range(lo, hi + 1)]

                z = z_tiles.pop(tau)
                lchunks = {}
                for l in range(lo, hi + 1):
                    ch = layer_chunks(l, tau - l, ht_prev)
                    if l == 0:
                        ch = ch[2:]      # x-part chunks pre-emitted in alloc_z
                        starts = [False] * len(ch)
                    else:
                        starts = [k == 0 for k in range(len(ch))]
                    lchunks[l] = [(lhsT, rhs, st, k == len(ch) - 1)
                                  for k, ((lhsT, rhs), st) in
                                  enumerate(zip(ch, starts))]
                maxk = max(len(v) for v in lchunks.values())
                for half in range(2):
                    ns = slice(HALF * half, HALF * (half + 1))
                    # interleave layers per chunk step: consecutive matmuls
                    # target different 32-col groups -> concurrent PE tiles
                    for k in range(maxk):
                        for l in range(lo, hi + 1):
                            chunks = lchunks[l]
                            if k >= len(chunks):
                                continue
                            lhsT, rhs, st, sp = chunks[k]
                            zl = z[32 * l:32 * (l + 1), ns]
                            nc.tensor.matmul(
                                zl, lhsT, rhs[:, ns],
                                start=st, stop=sp,
                                skip_group_check=True,
                            )

                gates = zg.tile([96, G4], FP32, name="gates", tag="gates")
                t1 = zg.tile([96, H], FP32, name="t1", tag="t1")
                th = zg.tile([96, H], FP32, name="th", tag="th")
                h_all = hpool.tile([96, H], BF16, name="h_all", tag="h_all")
                for r in rlist:
                    if has_bias:
                        nc.scalar.activation(gates[r, 0:768], z[r, 0:768],
                                             mybir.ActivationFunctionType.Sigmoid)
                    else:
                        nc.scalar.activation(gates[r, SL_F], z[r, SL_F],
                                             mybir.ActivationFunctionType.Sigmoid,
                                             bias=1.0)
                        nc.scalar.activation(gates[r, 256:768], z[r, 256:768],
                                             mybir.ActivationFunctionType.Sigmoid)
                    nc.scalar.activation(gates[r, SL_J], z[r, SL_J],
                                         mybir.ActivationFunctionType.Tanh)
                    nc.vector.tensor_tensor(c_all[r], gates[r, SL_F], c_all[r],
                                            op=mybir.AluOpType.mult)
                    nc.vector.tensor_tensor(t1[r], gates[r, SL_I],
                                            gates[r, SL_J],
                                            op=mybir.AluOpType.mult)
                    nc.vector.tensor_tensor(c_all[r], c_all[r], t1[r],
                                            op=mybir.AluOpType.add)
                    nc.scalar.activation(th[r], c_all[r],
                                         mybir.ActivationFunctionType.Tanh)
                    nc.vector.tensor_tensor(h_all[r], gates[r, SL_O], th[r],
                                            op=mybir.AluOpType.mult)
                if tau < 2:
                    # zero the not-yet-active layers' rows so their h^T reads
                    # as the correct zero initial state next tick
                    for rz in range(hi + 1, 3):
                        nc.vector.memset(h_all[32 * rz:32 * (rz + 1), :], 0.0)

                if tau + 1 <= T + 1:
                    alloc_z(tau + 1)

                ht = htp.tile([128, 2, 96], BF16, name="ht", tag="ht")
                for c in range(2):
                    tp = pht.tile([128, 96], BF16, name="htpp", tag="htpp")
                    nc.tensor.transpose(tp[:], h_all[:, 128 * c:128 * (c + 1)],
                                        id_bf[0:96, 0:96])
                    nc.vector.tensor_copy(ht[:, c, :], tp[:])

                if tau >= 2:
                    nc.vector.tensor_tensor(maxht[:], maxht[:], ht[:, :, 64:96],
                                            op=mybir.AluOpType.max)
                ht_prev = ht

        if not with_tail:
            # cost-model builds stop before the collective tail; keep maxht
            # live by dumping a slice to the output tensor
            nc.gpsimd.dma_start(out[0:6, 0:32], maxht[0:6, 0, :])
        else:
            # ---- AllGather of per-core maxes; dense head on every core ----
            tc.strict_bb_all_engine_barrier()
            mh_dram = dram.tile([128, 2 * BQ], BF16)
            nc.sync.dma_start(
                mh_dram[:].rearrange("p (c rr) -> p c rr", c=2), maxht[:, :, :])
            ag = dram.tile([8 * 128, 2 * BQ], BF16)
            nc.gpsimd.collective_compute(
                "AllGather",
                mybir.AluOpType.bypass,
                replica_groups=[list(range(8))],
                ins=[mh_dram[:].opt()],
                outs=[ag[:].opt()],
            )

            # rnn^T chunk (d2, c) [128, 128]: feature f = 256*d2 + 128*c + p,
            # batch b = 32*q + rr  ->  ag[(4*d2+q)*128 + p, c*32 + rr]
            tc.strict_bb_all_engine_barrier()
            agv = ag[:].rearrange("(g p) (c rr) -> g p c rr", p=128, c=2)
            rnn_chunks = []
            for d2 in range(2):
                for c in range(2):
                    rc = gpool.tile([128, 4, 32], BF16, name=f"rnn_{d2}_{c}",
                                    tag="rnn", bufs=4)
                    nc.sync.dma_start(
                        rc[:],
                        agv[4 * d2:4 * d2 + 4, :, c, :].rearrange("g p rr -> p g rr"))
                    rnn_chunks.append(rc)

            with tc.tile_pool(name="pdense", bufs=1, space="PSUM") as pdense:
                h1t = pdense.tile([64, B_FULL], FP32)
                for k in range(4):
                    nc.tensor.matmul(
                        h1t[:], d1w_sb[k][:],
                        rnn_chunks[k][:].rearrange("p g rr -> p (g rr)"),
                        start=(k == 0), stop=False, skip_group_check=True)
                nc.tensor.matmul(h1t[:], d1b_sb[:], ones_bf[:],
                                 start=False, stop=True, skip_group_check=True)

                # elu(x) = max(x,0) + exp(min(x,0)) - 1
                m = zg.tile([64, B_FULL], FP32, name="m", tag="m")
                nc.vector.tensor_scalar_min(m[:], h1t[:], 0.0)
                e = zg.tile([64, B_FULL], FP32, name="e", tag="m")
                nc.scalar.activation(e[:], m[:], mybir.ActivationFunctionType.Exp)
                h1f = zg.tile([64, B_FULL], FP32, name="h1f", tag="m")
                nc.vector.tensor_scalar_max(h1f[:], h1t[:], 0.0)
                nc.vector.tensor_tensor(h1f[:], h1f[:], e[:], op=mybir.AluOpType.add)
                nc.vector.tensor_scalar_add(h1f[:], h1f[:], -1.0)

                o_ps = pdense.tile([NC_OUT, B_FULL], FP32)
                nc.tensor.matmul(o_ps[:], d2w_sb[:], h1f[:], start=True, stop=False,
                                 skip_group_check=True)
                nc.tensor.matmul(o_ps[:], d2b_sb[:], ones_f32[:],
                                 start=False, stop=True, skip_group_check=True)
                o_sb = zg.tile([NC_OUT, B_FULL], FP32, name="o_sb", tag="m")
                nc.scalar.activation(o_sb[:], o_ps[:],
                                     mybir.ActivationFunctionType.Sigmoid)
                nc.sync.dma_start(out[:, :], o_sb[:])

    nc.finalize()
    return nc


_NC_CACHE = {}
TRACE = False
LAST_RESULTS = None
LAST_RUN_WALL_S = None


def _get_program(T, has_bias=True):
    key = (T, has_bias)
    if key not in _NC_CACHE:
        _NC_CACHE[key] = _build_program(T, has_bias=has_bias)
    return _NC_CACHE[key]


def _gate_perm():
    # TF order [i, j, f, o] (256 each) -> [f, i, o, j]
    i = np.arange(0, 256)
    j = np.arange(256, 512)
    f = np.arange(512, 768)
    o = np.arange(768, 1024)
    return np.concatenate([f, i, o, j])


def _prep_lstm_w(W, b, cap_table, perm, layer0, has_bias):
    """Gate-permute, fold cap_table (layer 0) and forget bias, add bias row.

    When has_bias is False the +1.0 forget bias is applied on-device via the
    ScalarE activation bias, and layers 1/2 carry no bias row at all."""
    Wp = np.asarray(W, np.float32)[:, perm]
    bp = np.asarray(b, np.float32)[perm].copy()
    if has_bias:
        bp[0:256] += 1.0  # forget_bias folded into the sigmoid argument
    if layer0:
        w_emb = Wp[0:200]
        w_cap = np.asarray(cap_table, np.float32) @ Wp[200:203]  # [4, 1024]
        w_h = Wp[203:459]
        stacked = np.concatenate(
            [w_emb[0:128], w_emb[128:200], w_cap, bp[None, :], w_h], axis=0)
        assert stacked.shape[0] == 461
    elif has_bias:
        stacked = np.concatenate([Wp[0:256], bp[None, :], Wp[256:512]], axis=0)
        assert stacked.shape[0] == 513
    else:
        stacked = Wp
        assert stacked.shape[0] == 512
    return stacked


def _to_bf16(x):
    import ml_dtypes
    return np.ascontiguousarray(np.asarray(x)).astype(ml_dtypes.bfloat16)


def kernel(**inputs):
    words = np.asarray(inputs["words"])
    capitals = np.asarray(inputs["capitals"])
    B, T = words.shape
    assert B == B_FULL

    perm = _gate_perm()
    cap_table = np.asarray(inputs["cap_table"], np.float32)
    # biases of layers 1/2 are usually all-zero; then the only bias is the
    # +1.0 forget bias, applied for free via the ScalarE activation bias,
    # and the per-step bias matmuls are dropped entirely
    hb = any(np.any(np.asarray(inputs[k], np.float32) != 0.0)
             for k in ("bf1", "bf2", "bb1", "bb2"))
    nc = _get_program(T, hb)

    w_by_dir = [
        [_prep_lstm_w(inputs["Wf0"], inputs["bf0"], cap_table, perm, True, hb),
         _prep_lstm_w(inputs["Wf1"], inputs["bf1"], cap_table, perm, False, hb),
         _prep_lstm_w(inputs["Wf2"], inputs["bf2"], cap_table, perm, False, hb)],
        [_prep_lstm_w(inputs["Wb0"], inputs["bb0"], cap_table, perm, True, hb),
         _prep_lstm_w(inputs["Wb1"], inputs["bb1"], cap_table, perm, False, hb),
         _prep_lstm_w(inputs["Wb2"], inputs["bb2"], cap_table, perm, False, hb)],
    ]
    w_bf = [[_to_bf16(w) for w in ws] for ws in w_by_dir]

    emb_np = np.ascontiguousarray(np.asarray(inputs["embed_words"], np.float32))
    d1w_np = _to_bf16(inputs["d1_W"])
    d1b_np = _to_bf16(np.asarray(inputs["d1_b"])[None, :])
    d2w_np = np.ascontiguousarray(np.asarray(inputs["d2_W"], np.float32))
    d2b_np = np.ascontiguousarray(np.asarray(inputs["d2_b"], np.float32)[None, :])

    in_maps = []
    for p in range(8):
        d, q = p // 4, p % 4
        wl = words[BQ * q:BQ * (q + 1)]
        cl = capitals[BQ * q:BQ * (q + 1)]
        if d == 1:
            wl = wl[:, ::-1]
            cl = cl[:, ::-1]
        # t-major token order r = t*BQ + b, fed as [128, NTILE], token = 128j+p
        wflat = np.ascontiguousarray(wl.T).reshape(-1)
        ntile = wflat.shape[0] // 128
        widx_np = np.ascontiguousarray(
            wflat.reshape(ntile, 128).T).astype(np.int32)
        cflat = np.ascontiguousarray(cl.T).reshape(-1)
        caph_np = _to_bf16(np.concatenate(
            [(cflat[None, :] == np.arange(4)[:, None]).astype(np.float32),
             np.ones((1, cflat.shape[0]), np.float32)], axis=0))

        in_maps.append({
            "widx": widx_np,
            "caph": caph_np,
            "emb": emb_np,
            "w0": w_bf[d][0],
            "w1": w_bf[d][1],
            "w2": w_bf[d][2],
            "d1w": d1w_np,
            "d1b": d1b_np,
            "d2w": d2w_np,
            "d2b": d2b_np,
        })

    global LAST_RESULTS, LAST_RUN_WALL_S
    import time as _time
    kwargs = {}
    if TRACE:
        kwargs = dict(trace=True, trace_cores=list(range(8)))
    _t0 = _time.time()
    try:
        res = run_bass_kernel_spmd(nc, in_maps, core_ids=list(range(8)), **kwargs)
    except Exception:
        if not kwargs:
            raise
        res = run_bass_kernel_spmd(nc, in_maps, core_ids=list(range(8)))
    LAST_RUN_WALL_S = _time.time() - _t0
    LAST_RESULTS = res
    return np.ascontiguousarray(res.results[0]["out"].T.astype(np.float32))



# revision 2
# speedup vs baseline: 1.5298x; 1.5298x over previous
"""BiLSTM Trainium2 kernel, v4 — For_i tick loop + fp8 uploads.

Same sharding + host-side gather/head as v2/v3 (8 cores = 4 batch quarters
x 2 directions, X^T built on host, no device collectives, dense head on
host), same For_i hardware tick loop as v3 (2 ticks per body).

v4 versus v3: X^T and the LSTM weights ship as float8 e4m3 (half the
bytes of bf16; the warm-call wall is dominated by input upload at
~55MB/s). Values are pre-scaled by 8 on host so the N(0, 0.05)-scale
entries sit in e4m3's normal range, then dequantized to bf16 on device
with exact power-of-two compensation:
  - X^T tiles convert fp8 -> bf16 with no rescale (they stay x8),
  - W0's x-facing chunks scale by 1/64 (their own 1/8 plus X^T's 1/8),
  - all other weight chunks scale by 1/8.
Offline fp32 simulation of this quantization gives max rel err ~1.2e-3
vs the fp32 reference (gate is 2e-2).

Structure:
  - peel ticks 0..3 (warmup: layer activation ramp + h zero-init)
  - For_i(iv = 128..TOK step 64): body = ticks (tau, tau+1), tau = iv/32:
      DMA xcur(fp8) <- X^T[:, iv:iv+64] (dynamic offset), convert to
      bf16, all compute at fixed addresses; ht ping-pong; PSUM z
      ping-pong (zE/zO); running maxht update
  - cooldown ticks 500, 501 (layers 1,2 then 2 only)
"""

import sys

import numpy as np

sys.path.insert(0, "/opt/trn_rl_repo")

from contextlib import ExitStack

import concourse.bacc as bacc
import concourse.mybir as mybir
import concourse.tile as tile
from concourse.bass import ds
from concourse.bass_utils import run_bass_kernel_spmd
from concourse.masks import make_identity

FP32 = mybir.dt.float32
BF16 = mybir.dt.bfloat16
FP8 = mybir.dt.float8e4

EMB, T_FULL, B_FULL, H, NC_OUT = 200, 500, 128, 256, 6
BQ = 32          # batch per core
G4 = 4 * H       # 1024 gate width
HALF = 512       # matmul N per PSUM bank
XTA = 128        # X^T rows in first chunk
XTB = 76         # X^T rows in second chunk (72 emb + 3 cap + 1 ones)

# gate slices after host permutation [f, i, o, j]
SL_F = slice(0, 256)
SL_I = slice(256, 512)
SL_O = slice(512, 768)
SL_J = slice(768, 1024)


def _build_program(T, has_bias=True):
    TOK = BQ * T
    assert T >= 8 and T % 2 == 0

    nc = bacc.Bacc(None, target_bir_lowering=False, debug=False)

    xta_d = nc.dram_tensor("xta", [XTA, TOK], FP8, kind="ExternalInput")
    xtb_d = nc.dram_tensor("xtb", [XTB, TOK], FP8, kind="ExternalInput")
    w0 = nc.dram_tensor("w0", [460, G4], FP8, kind="ExternalInput")
    wrows = 513 if has_bias else 512
    w1 = nc.dram_tensor("w1", [wrows, G4], FP8, kind="ExternalInput")
    w2 = nc.dram_tensor("w2", [wrows, G4], FP8, kind="ExternalInput")
    out = nc.dram_tensor("out", [128, 2 * BQ], BF16, kind="ExternalOutput")

    with tile.TileContext(nc) as tc, ExitStack() as ctx:
        const = ctx.enter_context(tc.tile_pool(name="const", bufs=1))
        wpool = ctx.enter_context(tc.tile_pool(name="wpool", bufs=1))
        xpool = ctx.enter_context(tc.tile_pool(name="xpool", bufs=1))
        state = ctx.enter_context(tc.tile_pool(name="state", bufs=1))
        work = ctx.enter_context(tc.tile_pool(name="work", bufs=1))

        # ---- constants ----
        id_bf = const.tile([128, 128], BF16)
        id_f32 = const.tile([128, 128], FP32)
        make_identity(nc, id_f32[:])
        nc.vector.tensor_copy(id_bf[:], id_f32[:])
        ones_bf = const.tile([1, 128], BF16)
        nc.gpsimd.memset(ones_bf[:], 1.0)

        # ---- weights -> SBUF: DMA fp8, dequant to bf16 with 2^-k scale ----
        # shipped values are true_W * 8; X^T stays *8 on device, so W0's
        # x-facing chunks (w0a, w0b) need an extra 1/8
        def load_w(dw, rows_chunks, scales):
            tiles = []
            r0 = 0
            for i, (rs, sc) in enumerate(zip(rows_chunks, scales)):
                t8 = wpool.tile([rs, G4], FP8, name=f"wt8_{dw.name}_{i}")
                nc.sync.dma_start(t8[:], dw[r0:r0 + rs, :])
                t = wpool.tile([rs, G4], BF16, name=f"wt_{dw.name}_{i}")
                nc.scalar.activation(t[:], t8[:],
                                     mybir.ActivationFunctionType.Copy,
                                     scale=sc)
                tiles.append(t)
                r0 += rs
            return tiles

        w0a, w0b, w0c, w0d = load_w(
            w0, [128, 76, 128, 128], [1 / 64, 1 / 64, 1 / 8, 1 / 8])
        if has_bias:
            w1a, w1b, w1bias, w1c, w1d = load_w(
                w1, [128, 128, 1, 128, 128], [1 / 8] * 5)
            w2a, w2b, w2bias, w2c, w2d = load_w(
                w2, [128, 128, 1, 128, 128], [1 / 8] * 5)
        else:
            w1a, w1b, w1c, w1d = load_w(
                w1, [128, 128, 128, 128], [1 / 8] * 4)
            w2a, w2b, w2c, w2d = load_w(
                w2, [128, 128, 128, 128], [1 / 8] * 4)
            w1bias = w2bias = None

        # ---- recurrent state (fixed addresses) ----
        c_all = state.tile([96, H], FP32)
        nc.gpsimd.memset(c_all[:], 0.0)
        maxht = state.tile([128, 2, BQ], BF16)
        nc.gpsimd.memset(maxht[:], -10.0)
        ht_a = state.tile([128, 2, 96], BF16)   # ht before even ticks
        nc.gpsimd.memset(ht_a[:], 0.0)
        ht_b = state.tile([128, 2, 96], BF16)   # ht before odd ticks

        # peel region X^T: ticks 0..3 use cols 0:128 (fp8 -> bf16, still x8)
        xp_a8 = xpool.tile([XTA, 4 * BQ], FP8)
        nc.sync.dma_start(xp_a8[:], xta_d[:, 0:4 * BQ])
        xp_b8 = xpool.tile([XTB, 4 * BQ], FP8)
        nc.sync.dma_start(xp_b8[:], xtb_d[:, 0:4 * BQ])
        xp_a = xpool.tile([XTA, 4 * BQ], BF16)
        nc.vector.tensor_copy(xp_a[:], xp_a8[:])
        xp_b = xpool.tile([XTB, 4 * BQ], BF16)
        nc.vector.tensor_copy(xp_b[:], xp_b8[:])

        # body X^T slices (refilled by DMA + converted each iteration)
        xc_a8 = xpool.tile([XTA, 2 * BQ], FP8)
        xc_b8 = xpool.tile([XTB, 2 * BQ], FP8)
        xc_a = xpool.tile([XTA, 2 * BQ], BF16)
        xc_b = xpool.tile([XTB, 2 * BQ], BF16)

        # per-parity work tiles (fixed addresses, reused every iteration)
        wt = {}
        for par in (0, 1):
            wt[par] = dict(
                gates=work.tile([96, G4], FP32, name=f"gates{par}"),
                t1=work.tile([96, H], FP32, name=f"t1_{par}"),
                th=work.tile([96, H], FP32, name=f"th_{par}"),
                h_all=work.tile([96, H], BF16, name=f"h_all{par}"),
            )

        def layer_chunks(l, ht, xa, xb):
            """lhsT/rhs chunk list for layer l; xa/xb are this tick's X^T APs."""
            if l == 0:
                return [
                    (xa, w0a),
                    (xb, w0b),
                    (ht[:, 0, 0:32], w0c),
                    (ht[:, 1, 0:32], w0d),
                ]
            wa, wb, wbias, wc, wd = (
                (w1a, w1b, w1bias, w1c, w1d) if l == 1 else
                (w2a, w2b, w2bias, w2c, w2d))
            xs = slice(32 * (l - 1), 32 * l)
            hs = slice(32 * l, 32 * (l + 1))
            chunks = [
                (ht[:, 0, xs], wa),
                (ht[:, 1, xs], wb),
                (ht[:, 0, hs], wc),
                (ht[:, 1, hs], wd),
            ]
            if has_bias:
                chunks.insert(2, (ones_bf[0:1, 0:32], wbias))
            return chunks

        def emit_x_parts(z, xa, xb):
            """L0 matmul chunks that depend only on X^T (not on ht)."""
            for half in range(2):
                ns = slice(HALF * half, HALF * (half + 1))
                for k, (lhsT, rhs) in enumerate(((xa, w0a), (xb, w0b))):
                    nc.tensor.matmul(z[0:32, ns], lhsT, rhs[:, ns],
                                     start=(k == 0), stop=False,
                                     skip_group_check=True)

        def emit_tick(*, lo, hi, z, xa, xb, ht_in, ht_out, par,
                      x_pre_emitted, zero_tail, do_max):
            """One wavefront tick: matmuls + gate math + h transpose."""
            if lo == 0:
                rlist = [slice(0, 32 * (hi + 1))]
            else:
                rlist = [slice(32 * l, 32 * (l + 1)) for l in range(lo, hi + 1)]

            lchunks = {}
            for l in range(lo, hi + 1):
                ch = layer_chunks(l, ht_in, xa, xb)
                if l == 0:
                    if x_pre_emitted:
                        ch = ch[2:]
                        starts = [False] * len(ch)
                    else:
                        starts = [k == 0 for k in range(len(ch))]
                else:
                    starts = [k == 0 for k in range(len(ch))]
                lchunks[l] = [(lhsT, rhs, st, k == len(ch) - 1)
                              for k, ((lhsT, rhs), st) in
                              enumerate(zip(ch, starts))]
            maxk = max(len(v) for v in lchunks.values())
            for half in range(2):
                ns = slice(HALF * half, HALF * (half + 1))
                for k in range(maxk):
                    for l in range(lo, hi + 1):
                        chunks = lchunks[l]
                        if k >= len(chunks):
                            continue
                        lhsT, rhs, st, sp = chunks[k]
                        nc.tensor.matmul(
                            z[32 * l:32 * (l + 1), ns], lhsT, rhs[:, ns],
                            start=st, stop=sp, skip_group_check=True)

            w = wt[par]
            gates, t1, th, h_all = w["gates"], w["t1"], w["th"], w["h_all"]
            for r in rlist:
                if has_bias:
                    nc.scalar.activation(gates[r, 0:768], z[r, 0:768],
                                         mybir.ActivationFunctionType.Sigmoid)
                else:
                    nc.scalar.activation(gates[r, SL_F], z[r, SL_F],
                                         mybir.ActivationFunctionType.Sigmoid,
                                         bias=1.0)
                    nc.scalar.activation(gates[r, 256:768], z[r, 256:768],
                                         mybir.ActivationFunctionType.Sigmoid)
                nc.scalar.activation(gates[r, SL_J], z[r, SL_J],
                                     mybir.ActivationFunctionType.Tanh)
                nc.vector.tensor_tensor(c_all[r], gates[r, SL_F], c_all[r],
                                        op=mybir.AluOpType.mult)
                nc.vector.tensor_tensor(t1[r], gates[r, SL_I], gates[r, SL_J],
                                        op=mybir.AluOpType.mult)
                nc.vector.tensor_tensor(c_all[r], c_all[r], t1[r],
                                        op=mybir.AluOpType.add)
                nc.scalar.activation(th[r], c_all[r],
                                     mybir.ActivationFunctionType.Tanh)
                nc.vector.tensor_tensor(h_all[r], gates[r, SL_O], th[r],
                                        op=mybir.AluOpType.mult)
            if zero_tail:
                for rz in range(hi + 1, 3):
                    nc.vector.memset(h_all[32 * rz:32 * (rz + 1), :], 0.0)

            for c in range(2):
                tp = pht.tile([128, 96], BF16, name=f"htpp{par}{c}",
                              tag=f"htpp{par}{c}")
                nc.tensor.transpose(tp[:], h_all[:, 128 * c:128 * (c + 1)],
                                    id_bf[0:96, 0:96])
                nc.vector.tensor_copy(ht_out[:, c, :], tp[:])

            if do_max:
                nc.vector.tensor_tensor(maxht[:], maxht[:],
                                        ht_out[:, :, 64:96],
                                        op=mybir.AluOpType.max)

        with tc.tile_pool(name="pz", bufs=1, space="PSUM") as pz, \
             tc.tile_pool(name="pht", bufs=1, space="PSUM") as pht:
            zE = pz.tile([96, G4], FP32, name="zE")
            zO = pz.tile([96, G4], FP32, name="zO")

            # ---- peel ticks 0..3 ----
            for tau in range(4):
                par = tau % 2
                z = (zE, zO)[par]
                ht_in, ht_out = ((ht_a, ht_b), (ht_b, ht_a))[par]
                xa = xp_a[:, BQ * tau:BQ * (tau + 1)]
                xb = xp_b[:, BQ * tau:BQ * (tau + 1)]
                emit_tick(lo=0, hi=min(2, tau), z=z, xa=xa, xb=xb,
                          ht_in=ht_in, ht_out=ht_out, par=par,
                          x_pre_emitted=False, zero_tail=(tau < 2),
                          do_max=(tau >= 2))

            # ---- hardware loop: ticks 4..T-1, two per iteration ----
            with tc.For_i(4 * BQ, TOK, 2 * BQ) as iv:
                nc.sync.dma_start(xc_a8[:], xta_d[:, ds(iv, 2 * BQ)])
                nc.sync.dma_start(xc_b8[:], xtb_d[:, ds(iv, 2 * BQ)])
                nc.vector.tensor_copy(xc_a[:], xc_a8[:])
                nc.vector.tensor_copy(xc_b[:], xc_b8[:])
                emit_x_parts(zE, xc_a[:, 0:BQ], xc_b[:, 0:BQ])
                emit_x_parts(zO, xc_a[:, BQ:2 * BQ], xc_b[:, BQ:2 * BQ])
                emit_tick(lo=0, hi=2, z=zE,
                          xa=xc_a[:, 0:BQ], xb=xc_b[:, 0:BQ],
                          ht_in=ht_a, ht_out=ht_b, par=0,
                          x_pre_emitted=True, zero_tail=False, do_max=True)
                emit_tick(lo=0, hi=2, z=zO,
                          xa=xc_a[:, BQ:2 * BQ], xb=xc_b[:, BQ:2 * BQ],
                          ht_in=ht_b, ht_out=ht_a, par=1,
                          x_pre_emitted=True, zero_tail=False, do_max=True)

            # ---- cooldown ticks T, T+1 (T is even: parity 0 then 1) ----
            for tau in (T, T + 1):
                par = tau % 2
                z = (zE, zO)[par]
                ht_in, ht_out = ((ht_a, ht_b), (ht_b, ht_a))[par]
                emit_tick(lo=tau - (T - 1), hi=2, z=z, xa=None, xb=None,
                          ht_in=ht_in, ht_out=ht_out, par=par,
                          x_pre_emitted=False, zero_tail=False, do_max=True)

        nc.sync.dma_start(
            out[:, :].rearrange("p (c rr) -> p c rr", c=2), maxht[:, :, :])

    nc.finalize()
    return nc


_NC_CACHE = {}
TRACE = False
LAST_RESULTS = None
LAST_RUN_WALL_S = None


def _get_program(T, has_bias=True):
    key = (T, has_bias)
    if key not in _NC_CACHE:
        _NC_CACHE[key] = _build_program(T, has_bias=has_bias)
    return _NC_CACHE[key]


def _gate_perm():
    # TF order [i, j, f, o] (256 each) -> [f, i, o, j]
    i = np.arange(0, 256)
    j = np.arange(256, 512)
    f = np.arange(512, 768)
    o = np.arange(768, 1024)
    return np.concatenate([f, i, o, j])


def _prep_lstm_w(W, b, perm, layer0, has_bias):
    Wp = np.asarray(W, np.float32)[:, perm]
    bp = np.asarray(b, np.float32)[perm].copy()
    if has_bias:
        # forget_bias folded into the sigmoid argument; in the no-bias path
        # the device's ScalarE bias=1.0 covers it for every layer
        bp[0:256] += 1.0
    if layer0:
        stacked = np.concatenate(
            [Wp[0:203], bp[None, :], Wp[203:459]], axis=0)
        assert stacked.shape[0] == 460
    elif has_bias:
        stacked = np.concatenate([Wp[0:256], bp[None, :], Wp[256:512]], axis=0)
        assert stacked.shape[0] == 513
    else:
        stacked = Wp
        assert stacked.shape[0] == 512
    return stacked


def _to_bf16(x):
    import ml_dtypes
    return np.ascontiguousarray(np.asarray(x)).astype(ml_dtypes.bfloat16)


def _elu(x):
    return np.where(x > 0, x, np.expm1(np.minimum(x, 0.0)))


def kernel(**inputs):
    import ml_dtypes

    words = np.asarray(inputs["words"])
    capitals = np.asarray(inputs["capitals"])
    B, T = words.shape
    assert B == B_FULL

    perm = _gate_perm()
    cap_table = np.asarray(inputs["cap_table"], np.float32)
    hb = any(np.any(np.asarray(inputs[k], np.float32) != 0.0)
             for k in ("bf1", "bf2", "bb1", "bb2"))
    nc = _get_program(T, hb)

    w_by_dir = [
        [_prep_lstm_w(inputs["Wf0"], inputs["bf0"], perm, True, hb),
         _prep_lstm_w(inputs["Wf1"], inputs["bf1"], perm, False, hb),
         _prep_lstm_w(inputs["Wf2"], inputs["bf2"], perm, False, hb)],
        [_prep_lstm_w(inputs["Wb0"], inputs["bb0"], perm, True, hb),
         _prep_lstm_w(inputs["Wb1"], inputs["bb1"], perm, False, hb),
         _prep_lstm_w(inputs["Wb2"], inputs["bb2"], perm, False, hb)],
    ]
    F8 = ml_dtypes.float8_e4m3
    # shipped weights are true * 8 in e4m3 (device rescales by 2^-3 / 2^-6)
    w_q8 = [[np.ascontiguousarray((np.asarray(w, np.float32) * 8).astype(F8))
             for w in ws] for ws in w_by_dir]

    # X^T in e4m3, pre-scaled by 8 (device dequant leaves the x8 in place
    # and folds the compensation into W0's x-facing chunks)
    emb_q8 = (np.asarray(inputs["embed_words"], np.float32) * 8).astype(F8)
    capt_q8 = (cap_table * 8).astype(F8)
    TOK = BQ * T
    xt_fw = []
    xt_bw = []
    for q in range(4):
        wl = words[BQ * q:BQ * (q + 1)]
        cl = capitals[BQ * q:BQ * (q + 1)]
        toks = wl.T.reshape(-1)
        caps = cl.T.reshape(-1)
        xt = np.empty((XTA + XTB, TOK), F8)
        xt[0:200] = emb_q8[toks].T
        xt[200:203] = capt_q8[caps].T
        xt[203] = np.float32(8.0)
        xt_fw.append(xt)
        xtb = np.ascontiguousarray(
            xt.reshape(XTA + XTB, T, BQ)[:, ::-1, :]).reshape(XTA + XTB, TOK)
        xt_bw.append(xtb)

    in_maps = []
    for p in range(8):
        d, q = p // 4, p % 4
        xt = (xt_fw, xt_bw)[d][q]
        in_maps.append({
            "xta": np.ascontiguousarray(xt[0:XTA]),
            "xtb": np.ascontiguousarray(xt[XTA:]),
            "w0": w_q8[d][0],
            "w1": w_q8[d][1],
            "w2": w_q8[d][2],
        })

    global LAST_RESULTS, LAST_RUN_WALL_S
    import time as _time
    kwargs = {}
    if TRACE:
        kwargs = dict(trace=True, trace_cores=list(range(8)))
    _t0 = _time.time()
    try:
        res = run_bass_kernel_spmd(nc, in_maps, core_ids=list(range(8)), **kwargs)
    except Exception:
        if not kwargs:
            raise
        res = run_bass_kernel_spmd(nc, in_maps, core_ids=list(range(8)))
    LAST_RUN_WALL_S = _time.time() - _t0
    LAST_RESULTS = res

    rnn_out = np.empty((B_FULL, 2 * H), np.float32)
    for p in range(8):
        d, q = p // 4, p % 4
        mh = np.asarray(res.results[p]["out"]).astype(np.float32)
        mh = mh.reshape(128, 2, BQ)
        for c in range(2):
            rnn_out[BQ * q:BQ * (q + 1),
                    256 * d + 128 * c:256 * d + 128 * (c + 1)] = mh[:, c, :].T
    d1_W = np.asarray(inputs["d1_W"], np.float32)
    d1_b = np.asarray(inputs["d1_b"], np.float32)
    d2_W = np.asarray(inputs["d2_W"], np.float32)
    d2_b = np.asarray(inputs["d2_b"], np.float32)
    h1 = _elu(rnn_out @ d1_W + d1_b)
    out = 1.0 / (1.0 + np.exp(-(h1 @ d2_W + d2_b)))
    return out.astype(np.float32)


# revision 3
# speedup vs baseline: 1.8629x; 1.2177x over previous
"""BiLSTM Trainium2 kernel, v6 — device-side gather from a sharded table.

Sharding: 8 cores = 4 batch quarters x 2 directions (as v4: 32 batch rows
per core, one direction each, 3-layer 96-row wavefront, For_i tick loop,
host-side dense head).

v6 versus v4: instead of shipping each core its pre-gathered X^T
(3.2MB fp8), ship the fp8 embedding table SHARDED over the 8 cores
(1.25MB/core) plus per-core token ids (64KB) and cap-feature rows (64KB),
and rebuild X^T on device: AllGather the table into DRAM, indirect-DMA
gather the 16000 token rows, PE-transpose to feature-major bf16, write to
a DRAM X^T buffer that the (unchanged) For_i tick loop streams from.
The weights also ship sharded by direction (0.37MB/core, AllGather over
each direction's 4 cores). Per-core upload drops ~6.1MB -> ~1.75MB; the
warm wall is upload-bound at ~55MB/s.

Numerics as v4: fp8 e4m3 uploads pre-scaled by 8, dequantized to bf16 on
device (X^T stays x8; W0's x-facing chunks carry 1/64, other chunks 1/8).
"""

import sys

import numpy as np

sys.path.insert(0, "/opt/trn_rl_repo")

from contextlib import ExitStack

import concourse.bacc as bacc
import concourse.mybir as mybir
import concourse.tile as tile
from concourse.bass import IndirectOffsetOnAxis, ds, ts
from concourse.bass_utils import run_bass_kernel_spmd
from concourse.masks import make_identity

FP32 = mybir.dt.float32
BF16 = mybir.dt.bfloat16
FP8 = mybir.dt.float8e4
INT32 = mybir.dt.int32

VOCAB, EMB, T_FULL, B_FULL, H, NC_OUT = 50000, 200, 500, 128, 256, 6
BQ = 32          # batch per core
G4 = 4 * H       # 1024 gate width
HALF = 512       # matmul N per PSUM bank
XTA = 128        # X^T rows in first chunk
XTB = 76         # X^T rows in second chunk (72 emb + 3 cap + 1 ones)
ESH = VOCAB // 8  # embedding-table rows per core
WSH = 372        # weight-shard rows per core (4 * 372 = 1488 per direction)

# gate slices after host permutation [f, i, o, j]
SL_F = slice(0, 256)
SL_I = slice(256, 512)
SL_O = slice(512, 768)
SL_J = slice(768, 1024)


def _build_program(T, esh, has_bias=True):
    """esh = embedding-shard rows per core (compacted vocab / 8)."""
    TOK = BQ * T
    NTILE = TOK // 128
    assert T >= 8 and T % 2 == 0 and TOK % 128 == 0

    nc = bacc.Bacc(None, target_bir_lowering=False, debug=False)

    embsh = nc.dram_tensor("embsh", [esh, EMB], FP8, kind="ExternalInput")
    widx = nc.dram_tensor("widx", [128, NTILE], INT32, kind="ExternalInput")
    caph = nc.dram_tensor("caph", [4, TOK], FP8, kind="ExternalInput")
    wcat = nc.dram_tensor("wcat", [WSH, G4], FP8, kind="ExternalInput")
    out = nc.dram_tensor("out", [128, 2 * BQ], BF16, kind="ExternalOutput")

    wr = 513 if has_bias else 512
    assert 460 + 2 * wr <= 4 * WSH

    with tile.TileContext(nc) as tc, ExitStack() as ctx:
        dram = ctx.enter_context(tc.tile_pool(name="dram", bufs=1,
                                              space="DRAM"))
        const = ctx.enter_context(tc.tile_pool(name="const", bufs=1))
        wpool = ctx.enter_context(tc.tile_pool(name="wpool", bufs=1))
        xpool = ctx.enter_context(tc.tile_pool(name="xpool", bufs=1))
        state = ctx.enter_context(tc.tile_pool(name="state", bufs=1))
        work = ctx.enter_context(tc.tile_pool(name="work", bufs=1))
        gpool = ctx.enter_context(tc.tile_pool(name="gpool", bufs=3))

        # ---- collectives: reconstruct emb table + direction weights ----
        # (collectives cannot read IO tensors directly; stage via DRAM tiles)
        emb_stage = dram.tile([esh, EMB], FP8)
        nc.sync.dma_start(emb_stage[:], embsh[:, :])
        w_stage = dram.tile([WSH, G4], FP8)
        nc.sync.dma_start(w_stage[:], wcat[:, :])
        tc.strict_bb_all_engine_barrier()
        emb_full = dram.tile([8 * esh, EMB], FP8)
        nc.gpsimd.collective_compute(
            "AllGather",
            mybir.AluOpType.bypass,
            replica_groups=[list(range(8))],
            ins=[emb_stage[:].opt()],
            outs=[emb_full[:].opt()],
        )
        wfull = dram.tile([4 * WSH, G4], FP8)
        nc.gpsimd.collective_compute(
            "AllGather",
            mybir.AluOpType.bypass,
            replica_groups=[[0, 1, 2, 3], [4, 5, 6, 7]],
            ins=[w_stage[:].opt()],
            outs=[wfull[:].opt()],
        )
        tc.strict_bb_all_engine_barrier()

        # X^T staging buffer in DRAM (fp8, x8-scaled), filled by the gather
        xt_dram = dram.tile([XTA + XTB, TOK], FP8)

        # ---- constants ----
        id_bf = const.tile([128, 128], BF16)
        id_f32 = const.tile([128, 128], FP32)
        make_identity(nc, id_f32[:])
        nc.vector.tensor_copy(id_bf[:], id_f32[:])
        ones_bf = const.tile([1, 128], BF16)
        nc.gpsimd.memset(ones_bf[:], 1.0)

        widx_sb = const.tile([128, NTILE], INT32)
        nc.sync.dma_start(widx_sb[:], widx[:, :])

        # ---- weights -> SBUF: DMA fp8 rows from wfull, dequant to bf16 ----
        def load_w(r0, rows_chunks, scales, nm):
            tiles = []
            for i, (rs, sc) in enumerate(zip(rows_chunks, scales)):
                t8 = wpool.tile([rs, G4], FP8, name=f"wt8_{nm}_{i}")
                nc.sync.dma_start(t8[:], wfull[r0:r0 + rs, :])
                t = wpool.tile([rs, G4], BF16, name=f"wt_{nm}_{i}")
                nc.scalar.activation(t[:], t8[:],
                                     mybir.ActivationFunctionType.Copy,
                                     scale=sc)
                tiles.append(t)
                r0 += rs
            return tiles

        w0a, w0b, w0c, w0d = load_w(
            0, [128, 76, 128, 128], [1 / 64, 1 / 64, 1 / 8, 1 / 8], "w0")
        if has_bias:
            w1a, w1b, w1bias, w1c, w1d = load_w(
                460, [128, 128, 1, 128, 128], [1 / 8] * 5, "w1")
            w2a, w2b, w2bias, w2c, w2d = load_w(
                460 + wr, [128, 128, 1, 128, 128], [1 / 8] * 5, "w2")
        else:
            w1a, w1b, w1c, w1d = load_w(
                460, [128, 128, 128, 128], [1 / 8] * 4, "w1")
            w2a, w2b, w2c, w2d = load_w(
                460 + wr, [128, 128, 128, 128], [1 / 8] * 4, "w2")
            w1bias = w2bias = None

        # ---- cap/ones rows -> xt_dram[200:204, :] ----
        cap8 = xpool.tile([4, TOK], FP8)
        nc.sync.dma_start(cap8[:], caph[:, :])
        nc.sync.dma_start(xt_dram[200:204, :], cap8[:])

        # ---- embedding gather + transpose into xt_dram[0:200, :] ----
        # (the DGE offset table must be a physical AP, so this loop is
        # python-unrolled; fp8 transpose + DMA-from-PSUM keep it at 5
        # instructions per 128-token tile)
        id_f8 = const.tile([128, 128], FP8)
        nc.vector.tensor_copy(id_f8[:], id_f32[:])
        with tc.tile_pool(name="pprep", bufs=2, space="PSUM") as pprep:
            for j in range(NTILE):
                g8 = gpool.tile([128, EMB], FP8, name="g8", tag="g8")
                nc.gpsimd.indirect_dma_start(
                    out=g8[:],
                    out_offset=None,
                    in_=emb_full[:, :],
                    in_offset=IndirectOffsetOnAxis(ap=widx_sb[:, j:j + 1],
                                                   axis=0),
                )
                gb = gpool.tile([128, EMB], BF16, name="gb", tag="gb")
                nc.vector.tensor_copy(gb[:], g8[:])
                tp1 = pprep.tile([128, 128], BF16, name="tp1", tag="tp")
                nc.tensor.transpose(tp1[:], gb[:, 0:128], id_bf[:])
                s1 = gpool.tile([128, 128], FP8, name="s1", tag="s1")
                nc.vector.tensor_copy(s1[:], tp1[:])
                nc.sync.dma_start(xt_dram[0:128, 128 * j:128 * (j + 1)],
                                  s1[:])
                tp2 = pprep.tile([72, 128], BF16, name="tp2", tag="tp")
                nc.tensor.transpose(tp2[:], gb[:, 128:200], id_bf[:])
                s2 = gpool.tile([72, 128], FP8, name="s2", tag="s2")
                nc.vector.tensor_copy(s2[:], tp2[:])
                nc.sync.dma_start(xt_dram[128:200, 128 * j:128 * (j + 1)],
                                  s2[:])

        tc.strict_bb_all_engine_barrier()

        # ---- recurrent state (fixed addresses) ----
        c_all = state.tile([96, H], FP32)
        nc.gpsimd.memset(c_all[:], 0.0)
        maxht = state.tile([128, 2, BQ], BF16)
        nc.gpsimd.memset(maxht[:], -10.0)
        ht_a = state.tile([128, 2, 96], BF16)   # ht before even ticks
        nc.gpsimd.memset(ht_a[:], 0.0)
        ht_b = state.tile([128, 2, 96], BF16)   # ht before odd ticks

        # peel region X^T: ticks 0..3 use cols 0:128 (fp8 -> bf16, still x8)
        xp_a8 = xpool.tile([XTA, 4 * BQ], FP8)
        nc.sync.dma_start(xp_a8[:], xt_dram[0:XTA, 0:4 * BQ])
        xp_b8 = xpool.tile([XTB, 4 * BQ], FP8)
        nc.sync.dma_start(xp_b8[:], xt_dram[XTA:XTA + XTB, 0:4 * BQ])
        xp_a = xpool.tile([XTA, 4 * BQ], BF16)
        nc.vector.tensor_copy(xp_a[:], xp_a8[:])
        xp_b = xpool.tile([XTB, 4 * BQ], BF16)
        nc.vector.tensor_copy(xp_b[:], xp_b8[:])

        # body X^T slices (refilled by DMA + converted each iteration)
        xc_a8 = xpool.tile([XTA, 2 * BQ], FP8)
        xc_b8 = xpool.tile([XTB, 2 * BQ], FP8)
        xc_a = xpool.tile([XTA, 2 * BQ], BF16)
        xc_b = xpool.tile([XTB, 2 * BQ], BF16)

        # per-parity work tiles (fixed addresses, reused every iteration)
        wt = {}
        for par in (0, 1):
            wt[par] = dict(
                gates=work.tile([96, G4], FP32, name=f"gates{par}"),
                t1=work.tile([96, H], FP32, name=f"t1_{par}"),
                th=work.tile([96, H], FP32, name=f"th_{par}"),
                h_all=work.tile([96, H], BF16, name=f"h_all{par}"),
            )

        def layer_chunks(l, ht, xa, xb):
            if l == 0:
                return [
                    (xa, w0a),
                    (xb, w0b),
                    (ht[:, 0, 0:32], w0c),
                    (ht[:, 1, 0:32], w0d),
                ]
            wa, wb, wbias, wc, wd = (
                (w1a, w1b, w1bias, w1c, w1d) if l == 1 else
                (w2a, w2b, w2bias, w2c, w2d))
            xs = slice(32 * (l - 1), 32 * l)
            hs = slice(32 * l, 32 * (l + 1))
            chunks = [
                (ht[:, 0, xs], wa),
                (ht[:, 1, xs], wb),
                (ht[:, 0, hs], wc),
                (ht[:, 1, hs], wd),
            ]
            if has_bias:
                chunks.insert(2, (ones_bf[0:1, 0:32], wbias))
            return chunks

        def emit_x_parts(z, xa, xb):
            for half in range(2):
                ns = slice(HALF * half, HALF * (half + 1))
                for k, (lhsT, rhs) in enumerate(((xa, w0a), (xb, w0b))):
                    nc.tensor.matmul(z[0:32, ns], lhsT, rhs[:, ns],
                                     start=(k == 0), stop=False,
                                     skip_group_check=True)

        def emit_tick(*, lo, hi, z, xa, xb, ht_in, ht_out, par,
                      x_pre_emitted, zero_tail, do_max):
            if lo == 0:
                rlist = [slice(0, 32 * (hi + 1))]
            else:
                rlist = [slice(32 * l, 32 * (l + 1)) for l in range(lo, hi + 1)]

            lchunks = {}
            for l in range(lo, hi + 1):
                ch = layer_chunks(l, ht_in, xa, xb)
                if l == 0:
                    if x_pre_emitted:
                        ch = ch[2:]
                        starts = [False] * len(ch)
                    else:
                        starts = [k == 0 for k in range(len(ch))]
                else:
                    starts = [k == 0 for k in range(len(ch))]
                lchunks[l] = [(lhsT, rhs, st, k == len(ch) - 1)
                              for k, ((lhsT, rhs), st) in
                              enumerate(zip(ch, starts))]
            maxk = max(len(v) for v in lchunks.values())
            for half in range(2):
                ns = slice(HALF * half, HALF * (half + 1))
                for k in range(maxk):
                    for l in range(lo, hi + 1):
                        chunks = lchunks[l]
                        if k >= len(chunks):
                            continue
                        lhsT, rhs, st, sp = chunks[k]
                        nc.tensor.matmul(
                            z[32 * l:32 * (l + 1), ns], lhsT, rhs[:, ns],
                            start=st, stop=sp, skip_group_check=True)

            w = wt[par]
            gates, t1, th, h_all = w["gates"], w["t1"], w["th"], w["h_all"]
            for r in rlist:
                if has_bias:
                    nc.scalar.activation(gates[r, 0:768], z[r, 0:768],
                                         mybir.ActivationFunctionType.Sigmoid)
                else:
                    nc.scalar.activation(gates[r, SL_F], z[r, SL_F],
                                         mybir.ActivationFunctionType.Sigmoid,
                                         bias=1.0)
                    nc.scalar.activation(gates[r, 256:768], z[r, 256:768],
                                         mybir.ActivationFunctionType.Sigmoid)
                nc.scalar.activation(gates[r, SL_J], z[r, SL_J],
                                     mybir.ActivationFunctionType.Tanh)
                nc.vector.tensor_tensor(c_all[r], gates[r, SL_F], c_all[r],
                                        op=mybir.AluOpType.mult)
                nc.vector.tensor_tensor(t1[r], gates[r, SL_I], gates[r, SL_J],
                                        op=mybir.AluOpType.mult)
                nc.vector.tensor_tensor(c_all[r], c_all[r], t1[r],
                                        op=mybir.AluOpType.add)
                nc.scalar.activation(th[r], c_all[r],
                                     mybir.ActivationFunctionType.Tanh)
                nc.vector.tensor_tensor(h_all[r], gates[r, SL_O], th[r],
                                        op=mybir.AluOpType.mult)
            if zero_tail:
                for rz in range(hi + 1, 3):
                    nc.vector.memset(h_all[32 * rz:32 * (rz + 1), :], 0.0)

            for c in range(2):
                tp = pht.tile([128, 96], BF16, name=f"htpp{par}{c}",
                              tag=f"htpp{par}{c}")
                nc.tensor.transpose(tp[:], h_all[:, 128 * c:128 * (c + 1)],
                                    id_bf[0:96, 0:96])
                nc.vector.tensor_copy(ht_out[:, c, :], tp[:])

            if do_max:
                nc.vector.tensor_tensor(maxht[:], maxht[:],
                                        ht_out[:, :, 64:96],
                                        op=mybir.AluOpType.max)

        with tc.tile_pool(name="pz", bufs=1, space="PSUM") as pz, \
             tc.tile_pool(name="pht", bufs=1, space="PSUM") as pht:
            zE = pz.tile([96, G4], FP32, name="zE")
            zO = pz.tile([96, G4], FP32, name="zO")

            # ---- peel ticks 0..3 ----
            for tau in range(4):
                par = tau % 2
                z = (zE, zO)[par]
                ht_in, ht_out = ((ht_a, ht_b), (ht_b, ht_a))[par]
                xa = xp_a[:, BQ * tau:BQ * (tau + 1)]
                xb = xp_b[:, BQ * tau:BQ * (tau + 1)]
                emit_tick(lo=0, hi=min(2, tau), z=z, xa=xa, xb=xb,
                          ht_in=ht_in, ht_out=ht_out, par=par,
                          x_pre_emitted=False, zero_tail=(tau < 2),
                          do_max=(tau >= 2))

            # ---- hardware loop: ticks 4..T-1, two per iteration ----
            with tc.For_i(4 * BQ, TOK, 2 * BQ) as iv:
                nc.sync.dma_start(xc_a8[:], xt_dram[0:XTA, ds(iv, 2 * BQ)])
                nc.sync.dma_start(xc_b8[:],
                                  xt_dram[XTA:XTA + XTB, ds(iv, 2 * BQ)])
                nc.vector.tensor_copy(xc_a[:], xc_a8[:])
                nc.vector.tensor_copy(xc_b[:], xc_b8[:])
                emit_x_parts(zE, xc_a[:, 0:BQ], xc_b[:, 0:BQ])
                emit_x_parts(zO, xc_a[:, BQ:2 * BQ], xc_b[:, BQ:2 * BQ])
                emit_tick(lo=0, hi=2, z=zE,
                          xa=xc_a[:, 0:BQ], xb=xc_b[:, 0:BQ],
                          ht_in=ht_a, ht_out=ht_b, par=0,
                          x_pre_emitted=True, zero_tail=False, do_max=True)
                emit_tick(lo=0, hi=2, z=zO,
                          xa=xc_a[:, BQ:2 * BQ], xb=xc_b[:, BQ:2 * BQ],
                          ht_in=ht_b, ht_out=ht_a, par=1,
                          x_pre_emitted=True, zero_tail=False, do_max=True)

            # ---- cooldown ticks T, T+1 ----
            for tau in (T, T + 1):
                par = tau % 2
                z = (zE, zO)[par]
                ht_in, ht_out = ((ht_a, ht_b), (ht_b, ht_a))[par]
                emit_tick(lo=tau - (T - 1), hi=2, z=z, xa=None, xb=None,
                          ht_in=ht_in, ht_out=ht_out, par=par,
                          x_pre_emitted=False, zero_tail=False, do_max=True)

        nc.sync.dma_start(
            out[:, :].rearrange("p (c rr) -> p c rr", c=2), maxht[:, :, :])

    nc.finalize()
    return nc


_NC_CACHE = {}
TRACE = False
LAST_RESULTS = None
LAST_RUN_WALL_S = None


def _get_program(T, esh, has_bias=True):
    key = (T, esh, has_bias)
    if key not in _NC_CACHE:
        _NC_CACHE[key] = _build_program(T, esh, has_bias=has_bias)
    return _NC_CACHE[key]


def _gate_perm():
    # TF order [i, j, f, o] (256 each) -> [f, i, o, j]
    i = np.arange(0, 256)
    j = np.arange(256, 512)
    f = np.arange(512, 768)
    o = np.arange(768, 1024)
    return np.concatenate([f, i, o, j])


def _prep_lstm_w(W, b, perm, layer0, has_bias):
    Wp = np.asarray(W, np.float32)[:, perm]
    bp = np.asarray(b, np.float32)[perm].copy()
    if has_bias:
        # forget_bias folded into the sigmoid argument; in the no-bias path
        # the device's ScalarE bias=1.0 covers it for every layer
        bp[0:256] += 1.0
    if layer0:
        stacked = np.concatenate(
            [Wp[0:203], bp[None, :], Wp[203:459]], axis=0)
        assert stacked.shape[0] == 460
    elif has_bias:
        stacked = np.concatenate([Wp[0:256], bp[None, :], Wp[256:512]], axis=0)
        assert stacked.shape[0] == 513
    else:
        stacked = Wp
        assert stacked.shape[0] == 512
    return stacked


def _elu(x):
    return np.where(x > 0, x, np.expm1(np.minimum(x, 0.0)))


def kernel(**inputs):
    import ml_dtypes

    words = np.asarray(inputs["words"])
    capitals = np.asarray(inputs["capitals"])
    B, T = words.shape
    assert B == B_FULL

    perm = _gate_perm()
    cap_table = np.asarray(inputs["cap_table"], np.float32)
    hb = any(np.any(np.asarray(inputs[k], np.float32) != 0.0)
             for k in ("bf1", "bf2", "bb1", "bb2"))

    # vocab compaction: ship only embedding rows some core actually reads
    uniq = np.unique(words)                     # sorted unique token ids
    words_c = np.searchsorted(uniq, words).astype(np.int32)
    esh = -(-len(uniq) // 8)                    # shard rows per core
    nc = _get_program(T, esh, hb)

    F8 = ml_dtypes.float8_e4m3
    # per-direction weight stacks, shipped as 1/4 shards within each group
    w_by_dir = []
    for d, (k0, k1, k2) in enumerate(
            [("Wf0", "Wf1", "Wf2"), ("Wb0", "Wb1", "Wb2")]):
        b0, b1, b2 = ("bf0", "bf1", "bf2") if d == 0 else ("bb0", "bb1", "bb2")
        wall = np.concatenate([
            _prep_lstm_w(inputs[k0], inputs[b0], perm, True, hb),
            _prep_lstm_w(inputs[k1], inputs[b1], perm, False, hb),
            _prep_lstm_w(inputs[k2], inputs[b2], perm, False, hb),
        ], axis=0)
        pad = 4 * WSH - wall.shape[0]
        wall = np.concatenate([wall, np.zeros((pad, G4), np.float32)], axis=0)
        w_by_dir.append((wall * 8).astype(F8))

    emb_c = np.asarray(inputs["embed_words"], np.float32)[uniq]
    pad = 8 * esh - emb_c.shape[0]
    if pad:
        emb_c = np.concatenate(
            [emb_c, np.zeros((pad, EMB), np.float32)], axis=0)
    emb_q8 = (emb_c * 8).astype(F8)
    capt_q8 = (cap_table * 8).astype(F8)
    TOK = BQ * T

    in_maps = []
    for p in range(8):
        d, q = p // 4, p % 4
        wl = words_c[BQ * q:BQ * (q + 1)]
        cl = capitals[BQ * q:BQ * (q + 1)]
        if d == 1:
            wl = wl[:, ::-1]
            cl = cl[:, ::-1]
        wflat = np.ascontiguousarray(wl.T).reshape(-1)   # r = t*32 + b
        ntile = wflat.shape[0] // 128
        widx_np = np.ascontiguousarray(
            wflat.reshape(ntile, 128).T).astype(np.int32)
        cflat = cl.T.reshape(-1)
        caph_np = np.empty((4, TOK), F8)
        caph_np[0:3] = capt_q8[cflat].T
        caph_np[3] = np.float32(8.0)

        in_maps.append({
            "embsh": np.ascontiguousarray(emb_q8[esh * p:esh * (p + 1)]),
            "widx": widx_np,
            "caph": caph_np,
            "wcat": np.ascontiguousarray(
                w_by_dir[d][WSH * q:WSH * (q + 1)]),
        })

    global LAST_RESULTS, LAST_RUN_WALL_S
    import time as _time
    kwargs = {}
    if TRACE:
        kwargs = dict(trace=True, trace_cores=list(range(8)))
    _t0 = _time.time()
    try:
        res = run_bass_kernel_spmd(nc, in_maps, core_ids=list(range(8)), **kwargs)
    except Exception:
        if not kwargs:
            raise
        res = run_bass_kernel_spmd(nc, in_maps, core_ids=list(range(8)))
    LAST_RUN_WALL_S = _time.time() - _t0
    LAST_RESULTS = res

    rnn_out = np.empty((B_FULL, 2 * H), np.float32)
    for p in range(8):
        d, q = p // 4, p % 4
        mh = np.asarray(res.results[p]["out"]).astype(np.float32)
        mh = mh.reshape(128, 2, BQ)
        for c in range(2):
            rnn_out[BQ * q:BQ * (q + 1),
                    256 * d + 128 * c:256 * d + 128 * (c + 1)] = mh[:, c, :].T
    d1_W = np.asarray(inputs["d1_W"], np.float32)
    d1_b = np.asarray(inputs["d1_b"], np.float32)
    d2_W = np.asarray(inputs["d2_W"], np.float32)
    d2_b = np.asarray(inputs["d2_b"], np.float32)
    h1 = _elu(rnn_out @ d1_W + d1_b)
    out = 1.0 / (1.0 + np.exp(-(h1 @ d2_W + d2_b)))
    return out.astype(np.float32)
